# revision 2
# baseline (speedup 1.0000x reference)
"""Causal single-head attention on 8 Trainium2 NeuronCores.

Problem: x [4, 2048, 1024] fp32; Wq/Wk/Wv [1024, 1024] fp32.
  q/k/v = x @ W*; scores = q k^T / 32 (causal); out = softmax(scores) @ v.

Sharding: 8 cores = 4 batches x 2 roles. Within a batch, the 16
128-row q-blocks are split alternately: role r takes global blocks
g = 2j+r (j = 0..7) — this balances causal attention work between the
pair. Weight folding is done on the *query* side: with G = Wq Wk^T,
scores = (x_q G) x_kv^T, so each core projects Q~ = x_q @ G only for
its OWN 1024 q tokens (half the cost of projecting K for all 2048 kv
tokens) and contracts scores directly against raw x_kv^T columns.
The V projection is likewise folded past the attention contraction:
out = (attn @ x_kv) @ Wv, so attn@V runs against raw x rows and Wv is
applied to the [1024, 1024] normalized context afterwards. No
projection over the 2048 kv tokens happens anywhere — all per-core
matmul work is proportional to its 1024 q rows except the causal
score/context contractions themselves.

Input DMAs are chunked and ordered by first use (G block 0, then the
first half of x_q^T, ...) so the first Q~ matmul issues ~3.5 us in
instead of waiting ~35 us for the full input set (the DMA engines are
a serial resource at ~360 GB/s).

Each q-block's tail (normalize -> PE-transpose -> @Wv -> store) is
emitted one block late, software-pipelined under the next block's
score/context matmuls.

The program is SPMD-uniform: role differences live only in the
host-gathered inputs (xq = q-token columns of x^T in q-block order)
and in the [128, 256] mask applied to the last two kv blocks of each
padded row ([tril|zeros] for role 0, [ones|tril] for role 1).

Numerics: all matmuls in bf16 (inputs rounded on host) with fp32
PSUM accumulation; softmax in fp32 without max-subtraction (scores
are O(5), exp can't overflow), normalization deferred to after the
attn@V matmul. End-to-end max-abs error vs the fp32 reference is
~1e-2 of the output scale.
"""

import numpy as np
import ml_dtypes

import concourse.bass as bass
import concourse.bacc as bacc
import concourse.tile as tile
from concourse import mybir
from concourse.bass_utils import run_bass_kernel_spmd
from concourse.masks import make_identity

P = 128
D = 1024          # d_in
E = 1024          # d_out
T = 2048          # seq len
B = 4             # batch
DT = D // P       # 8 d-tiles
QB = 8            # q blocks per core
KVB = T // P      # 16 kv blocks
NCORES = 8

FP32 = mybir.dt.float32
BF16 = mybir.dt.bfloat16

_CACHED_NC = None


def _build(pmm_bufs=2, pt_bufs=2, pu_bufs=2, work_bufs=2):
    nc = bacc.Bacc(None, target_bir_lowering=False)
    # gq: G = Wq @ Wk^T pre-tiled on host as [p, out-dblock b, (d'tile t, e')]
    # so gq[:, b, t*128:(t+1)*128] is the lhsT for output block b, tile t.
    gq = nc.dram_tensor("gq", [P, DT, D], BF16, kind="ExternalInput")
    xq = nc.dram_tensor("xq", [D, QB * P], BF16, kind="ExternalInput")
    xt = nc.dram_tensor("xt", [D, T], BF16, kind="ExternalInput")
    xn = nc.dram_tensor("xn", [T, D], BF16, kind="ExternalInput")
    wv = nc.dram_tensor("wv", [D, E], BF16, kind="ExternalInput")
    mask = nc.dram_tensor("mask", [P, 2 * P], BF16, kind="ExternalInput")
    out = nc.dram_tensor("out", [QB * P, E], FP32, kind="ExternalOutput")

    xq_r = xq.rearrange("(dt p) t -> p dt t", p=P)
    xt_r = xt.rearrange("(dt p) t -> p dt t", p=P)
    xn_r = xn.rearrange("(tt p) d -> p tt d", p=P)
    wv_r = wv.rearrange("(dt p) e -> p dt e", p=P)

    with tile.TileContext(nc) as tc:
        with (
            tc.tile_pool(name="const", bufs=1) as const,
            tc.tile_pool(name="big", bufs=1) as big,
            tc.tile_pool(name="wpool", bufs=1) as wpool,
            tc.tile_pool(name="work", bufs=work_bufs) as work,
            tc.tile_pool(name="small", bufs=8) as small,
            tc.tile_pool(name="pmm", bufs=pmm_bufs, space="PSUM") as pmm,
            tc.tile_pool(name="pt", bufs=pt_bufs, space="PSUM") as pt,
            tc.tile_pool(name="pu", bufs=pu_bufs, space="PSUM") as pu,
        ):
            ident = const.tile([P, P], BF16)
            make_identity(nc, ident[:])

            XT = big.tile([P, DT, T], BF16)       # raw x^T, d-tile major
            XN = big.tile([P, KVB, D], BF16)      # raw x rows, kv-tile major
            QT = big.tile([P, DT, QB * P], BF16)  # Q~^T (computed on-chip)
            gq_sb = wpool.tile([P, DT, D], BF16, tag="gq")
            xq_sb = wpool.tile([P, DT, QB * P], BF16, tag="xq")
            wv_sb = wpool.tile([P, DT, E], BF16, tag="wv")
            mask_sb = const.tile([P, 2 * P], BF16)

            # ---- input DMAs, ordered by first use (serial DMA resource)
            nc.sync.dma_start(out=gq_sb[:, 0, :], in_=gq[:, 0, :])
            nc.sync.dma_start(out=xq_sb[:, :, 0:512], in_=xq_r[:, :, 0:512])
            for b_ in range(1, DT):
                nc.sync.dma_start(out=gq_sb[:, b_, :], in_=gq[:, b_, :])
            nc.sync.dma_start(out=xq_sb[:, :, 512:1024], in_=xq_r[:, :, 512:1024])
            nc.sync.dma_start(out=mask_sb[:], in_=mask[:, :])
            nc.sync.dma_start(out=XT[:, :, 0:1024], in_=xt_r[:, :, 0:1024])
            nc.sync.dma_start(out=XN[:, 0:KVB // 2, :], in_=xn_r[:, 0:KVB // 2, :])
            nc.sync.dma_start(out=wv_sb[:], in_=wv_r[:, :, :])
            nc.sync.dma_start(out=XT[:, :, 1024:2048], in_=xt_r[:, :, 1024:2048])
            nc.sync.dma_start(out=XN[:, KVB // 2:KVB, :], in_=xn_r[:, KVB // 2:KVB, :])

            # ---- Phase Q: Q~^T = G^T x_q^T over this core's 1024 q tokens
            for h in range(2):
                for b_ in range(DT):
                    ps = pmm.tile([P, 512], FP32, tag="mm")
                    for t in range(DT):
                        nc.tensor.matmul(ps[:], gq_sb[:, b_, t * P:(t + 1) * P],
                                         xq_sb[:, t, h * 512:(h + 1) * 512],
                                         start=(t == 0), stop=(t == DT - 1))
                    nc.scalar.copy(QT[:, b_, h * 512:(h + 1) * 512], ps[:])

            # ---- Phase C: attention per q block.
            # The per-block tail (normalize -> transpose -> @Wv -> store) is
            # emitted one block late so its DVE/ACT dependencies resolve
            # while the PE runs the next block's score/context matmuls.
            def emit_tail(U, sums, j):
                recip = small.tile([P, 1], FP32)
                nc.vector.reciprocal(recip[:], sums[:])
                c_sb = work.tile([P, D], BF16, tag="csb")
                for dh in range(2):
                    nc.vector.tensor_scalar_mul(c_sb[:, dh * 512:(dh + 1) * 512],
                                                U[:, dh * 512:(dh + 1) * 512],
                                                recip[:])
                ps_c = pt.tile([P, D], BF16, tag="pt")
                for i in range(DT):
                    nc.tensor.transpose(ps_c[:, i * P:(i + 1) * P],
                                        c_sb[:, i * P:(i + 1) * P], ident[:])
                ct_sb = work.tile([P, D], BF16, tag="ct")
                nc.scalar.copy(ct_sb[:], ps_c[:])
                out_sb = work.tile([P, E], FP32, tag="out")
                for eh in range(2):
                    ps_o = pmm.tile([P, 512], FP32, tag="mm")
                    for dt in range(DT):
                        nc.tensor.matmul(ps_o[:], ct_sb[:, dt * P:(dt + 1) * P],
                                         wv_sb[:, dt, eh * 512:(eh + 1) * 512],
                                         start=(dt == 0), stop=(dt == DT - 1))
                    nc.scalar.copy(out_sb[:, eh * 512:(eh + 1) * 512], ps_o[:])
                nc.sync.dma_start(out=out[j * P:(j + 1) * P, :], in_=out_sb[:])

            pending = None
            for j in range(QB):
                n_kb = 2 * j + 2          # padded kv blocks for this q block
                widths = [512] * ((j + 1) // 2) + ([256] if j % 2 == 0 else [])
                sums = small.tile([P, 1], FP32)
                nc.vector.memset(sums[:], 0.0)
                U = pu.tile([P, E], FP32, tag="pu")
                c0 = 0
                for ci, w in enumerate(widths):
                    last = (ci == len(widths) - 1)
                    ps_s = pmm.tile([P, 512], FP32, tag="mm")
                    for dt in range(DT):
                        nc.tensor.matmul(ps_s[:, :w], QT[:, dt, j * P:(j + 1) * P],
                                         XT[:, dt, c0:c0 + w],
                                         start=(dt == 0), stop=(dt == DT - 1))
                    exps = work.tile([P, 512], BF16, tag="exps")
                    nc.scalar.activation(exps[:, :w], ps_s[:, :w],
                                         mybir.ActivationFunctionType.Exp,
                                         scale=1.0 / 32.0)
                    if last:
                        nc.vector.tensor_mul(exps[:, w - 256:w],
                                             exps[:, w - 256:w], mask_sb[:])
                    csum = small.tile([P, 1], FP32)
                    nc.vector.tensor_reduce(csum[:], exps[:, :w],
                                            axis=mybir.AxisListType.X,
                                            op=mybir.AluOpType.add)
                    nc.vector.tensor_add(sums[:], sums[:], csum[:])

                    ps_t = pt.tile([P, 512], BF16, tag="pt")
                    nblk = w // P
                    for i in range(nblk):
                        nc.tensor.transpose(ps_t[:, i * P:(i + 1) * P],
                                            exps[:, i * P:(i + 1) * P], ident[:])
                    expsT = work.tile([P, 512], BF16, tag="expsT")
                    nc.scalar.copy(expsT[:, :w], ps_t[:, :w])
                    for i in range(nblk):
                        kb = c0 // P + i
                        for dh in range(2):
                            nc.tensor.matmul(U[:, dh * 512:(dh + 1) * 512],
                                             expsT[:, i * P:(i + 1) * P],
                                             XN[:, kb, dh * 512:(dh + 1) * 512],
                                             start=(kb == 0), stop=(kb == n_kb - 1))
                    c0 += w
                if pending is not None:
                    emit_tail(*pending)
                pending = (U, sums, j)
            if pending is not None:
                emit_tail(*pending)

    nc.compile()
    return nc


def _get_nc():
    global _CACHED_NC
    if _CACHED_NC is None:
        _CACHED_NC = _build()
    return _CACHED_NC


def _prep_inputs(x, Wq, Wk, Wv):
    bf = ml_dtypes.bfloat16
    tril = np.tril(np.ones((P, P), np.float32))
    ones = np.ones((P, P), np.float32)
    zeros = np.zeros((P, P), np.float32)
    # fold Wq/Wk into the query projection: scores = (x_q G) x_kv^T with
    # G = Wq @ Wk^T, so only this core's 1024 q tokens get projected and
    # raw x_kv columns serve as the K side of the score matmul.
    G = (np.asarray(Wq, np.float64) @ np.asarray(Wk, np.float64).T)
    G = G.astype(np.float32).astype(bf)
    # [p, b, (t, e')] tiling: gq[p, b, t*128+e'] = G[t*128+p, b*128+e']
    gq_b = np.ascontiguousarray(
        G.reshape(DT, P, DT, P).transpose(1, 2, 0, 3).reshape(P, DT, D))
    wv_b = np.asarray(Wv, np.float32).astype(bf)
    in_maps = []
    for core in range(NCORES):
        b, r = core // 2, core % 2
        xtc = np.ascontiguousarray(x[b].T.astype(np.float32)).astype(bf)
        xqc = np.ascontiguousarray(
            xtc.reshape(D, KVB, P)[:, r::2, :].reshape(D, QB * P))
        m = (np.concatenate([tril, zeros], axis=1) if r == 0
             else np.concatenate([ones, tril], axis=1)).astype(bf)
        in_maps.append({
            "gq": gq_b,
            "xq": xqc,
            "xt": xtc,
            "xn": np.ascontiguousarray(x[b].astype(np.float32)).astype(bf),
            "wv": wv_b,
            "mask": m,
        })
    return in_maps


def _assemble(results, x_shape):
    outp = np.empty(x_shape, np.float32)
    for core in range(NCORES):
        b, r = core // 2, core % 2
        co = results[core]["out"]
        for j in range(QB):
            g = 2 * j + r
            outp[b, g * P:(g + 1) * P, :] = co[j * P:(j + 1) * P, :]
    return outp


def kernel(x, Wq, Wk, Wv):
    assert x.shape == (B, T, D) and Wq.shape == (D, E)
    nc = _get_nc()
    in_maps = _prep_inputs(x, Wq, Wk, Wv)
    res = run_bass_kernel_spmd(nc, in_maps, core_ids=list(range(NCORES)))
    return _assemble(res.results, x.shape)


# revision 7
# speedup vs baseline: 1.1330x; 1.1330x over previous
"""Causal single-head attention on 8 Trainium2 NeuronCores.

Problem: x [4, 2048, 1024] fp32; Wq/Wk/Wv [1024, 1024] fp32.
  q/k/v = x @ W*; scores = q k^T / 32 (causal); out = softmax(scores) @ v.

Sharding: 8 cores = 4 batches x 2 roles. Within a batch, the 16
128-row q-blocks are split alternately: role r takes global blocks
g = 2j+r (j = 0..7) — this balances causal attention work between the
pair. Weight folding is done on the *query* side: with G = Wq Wk^T,
scores = (x_q G) x_kv^T, so each core projects Q~ = x_q @ G only for
its OWN 1024 q tokens (half the cost of projecting K for all 2048 kv
tokens) and contracts scores directly against raw x_kv^T columns.
The V projection is likewise folded past the attention contraction:
out = (attn @ x_kv) @ Wv, so attn@V runs against raw x rows and Wv is
applied to the [1024, 1024] normalized context afterwards. No
projection over the 2048 kv tokens happens anywhere — all per-core
matmul work is proportional to its 1024 q rows except the causal
score/context contractions themselves.

Input DMAs are chunked and ordered by first use (G block 0, then the
first half of x_q^T, ...) so the first Q~ matmul issues ~3.5 us in
instead of waiting ~35 us for the full input set (the DMA engines are
a serial resource at ~360 GB/s).

Each q-block's tail (normalize -> PE-transpose -> @Wv -> store) is
emitted one block late, software-pipelined under the next block's
score/context matmuls.

The program is SPMD-uniform: role differences live only in the
host-gathered inputs (xq = q-token columns of x^T in q-block order)
and in the [128, 256] mask applied to the last two kv blocks of each
padded row ([tril|zeros] for role 0, [ones|tril] for role 1).

Numerics: all matmuls in bf16 (inputs rounded on host) with fp32
PSUM accumulation; softmax in fp32 without max-subtraction (scores
are O(5), exp can't overflow), normalization deferred to after the
attn@V matmul. End-to-end max-abs error vs the fp32 reference is
~1e-2 of the output scale.
"""

import numpy as np
import ml_dtypes

import concourse.bass as bass
import concourse.bacc as bacc
import concourse.tile as tile
from concourse import mybir
from concourse.bass_utils import run_bass_kernel_spmd
from concourse.masks import make_identity

P = 128
D = 1024          # d_in
E = 1024          # d_out
T = 2048          # seq len
B = 4             # batch
DT = D // P       # 8 d-tiles
QB = 8            # q blocks per core
KVB = T // P      # 16 kv blocks
NCORES = 8

FP32 = mybir.dt.float32
BF16 = mybir.dt.bfloat16

_CACHED_NC = None


def _build(pmm_bufs=2, pt_bufs=2, pu_bufs=2, work_bufs=2):
    nc = bacc.Bacc(None, target_bir_lowering=False)
    # gq: G = Wq @ Wk^T pre-tiled on host as [p, out-dblock b, (d'tile t, e')]
    # so gq[:, b, t*128:(t+1)*128] is the lhsT for output block b, tile t.
    gq = nc.dram_tensor("gq", [P, DT, D], BF16, kind="ExternalInput")
    xq = nc.dram_tensor("xq", [D, QB * P], BF16, kind="ExternalInput")
    xt = nc.dram_tensor("xt", [D, T], BF16, kind="ExternalInput")
    xn = nc.dram_tensor("xn", [T, D], BF16, kind="ExternalInput")
    wv = nc.dram_tensor("wv", [D, E], BF16, kind="ExternalInput")
    mask = nc.dram_tensor("mask", [P, 2 * P], BF16, kind="ExternalInput")
    out = nc.dram_tensor("out", [QB * P, E], FP32, kind="ExternalOutput")

    xq_r = xq.rearrange("(dt p) t -> p dt t", p=P)
    xt_r = xt.rearrange("(dt p) t -> p dt t", p=P)
    xn_r = xn.rearrange("(tt p) d -> p tt d", p=P)
    wv_r = wv.rearrange("(dt p) e -> p dt e", p=P)

    with tile.TileContext(nc) as tc:
        with (
            tc.tile_pool(name="const", bufs=1) as const,
            tc.tile_pool(name="big", bufs=1) as big,
            tc.tile_pool(name="wpool", bufs=1) as wpool,
            tc.tile_pool(name="work", bufs=work_bufs) as work,
            tc.tile_pool(name="small", bufs=8) as small,
            tc.tile_pool(name="pmm", bufs=pmm_bufs, space="PSUM") as pmm,
            tc.tile_pool(name="pt", bufs=pt_bufs, space="PSUM") as pt,
            tc.tile_pool(name="pu", bufs=pu_bufs, space="PSUM") as pu,
        ):
            ident = const.tile([P, P], BF16)
            make_identity(nc, ident[:])

            XT = big.tile([P, DT, T], BF16)       # raw x^T, d-tile major
            XN = big.tile([P, KVB, D], BF16)      # raw x rows, kv-tile major
            QT = big.tile([P, DT, QB * P], BF16)  # Q~^T (computed on-chip)
            gq_sb = wpool.tile([P, DT, D], BF16, tag="gq")
            xq_sb = wpool.tile([P, DT, QB * P], BF16, tag="xq")
            wv_sb = wpool.tile([P, DT, E], BF16, tag="wv")
            mask_sb = const.tile([P, 2 * P], BF16)

            # ---- input DMAs, ordered by first use (serial DMA resource)
            nc.sync.dma_start(out=gq_sb[:, 0, :], in_=gq[:, 0, :])
            nc.sync.dma_start(out=xq_sb[:, :, 0:512], in_=xq_r[:, :, 0:512])
            for b_ in range(1, DT):
                nc.sync.dma_start(out=gq_sb[:, b_, :], in_=gq[:, b_, :])
            nc.sync.dma_start(out=xq_sb[:, :, 512:1024], in_=xq_r[:, :, 512:1024])
            nc.sync.dma_start(out=mask_sb[:], in_=mask[:, :])
            nc.sync.dma_start(out=XT[:, :, 0:1024], in_=xt_r[:, :, 0:1024])
            nc.sync.dma_start(out=XN[:, 0:KVB // 2, :], in_=xn_r[:, 0:KVB // 2, :])
            nc.sync.dma_start(out=wv_sb[:], in_=wv_r[:, :, :])
            nc.sync.dma_start(out=XT[:, :, 1024:2048], in_=xt_r[:, :, 1024:2048])
            nc.sync.dma_start(out=XN[:, KVB // 2:KVB, :], in_=xn_r[:, KVB // 2:KVB, :])

            # ---- Phase Q: Q~^T = G^T x_q^T over this core's 1024 q tokens
            for h in range(2):
                for b_ in range(DT):
                    ps = pmm.tile([P, 512], FP32, tag="mm")
                    for t in range(DT):
                        nc.tensor.matmul(ps[:], gq_sb[:, b_, t * P:(t + 1) * P],
                                         xq_sb[:, t, h * 512:(h + 1) * 512],
                                         start=(t == 0), stop=(t == DT - 1))
                    nc.scalar.copy(QT[:, b_, h * 512:(h + 1) * 512], ps[:])

            # ---- Phase C: attention per q block.
            #
            # Context accumulation is "UT-direct": instead of U[q, d] (which
            # would need a PE transpose before the Wv matmul), accumulate
            # UT[d, q] = sum_kv XN[kv, d]^T expsT[kv, q] directly, so the Wv
            # matmul consumes it as lhsT with no transpose. Softmax
            # normalization commutes with the Wv matmul (per-q scalar) and is
            # fused into the final PSUM->SBUF copy as an ACT per-partition
            # scale.
            #
            # Pipelining: each chunk's context matmuls are emitted one chunk
            # late (under the next chunk's score matmuls), and each block's
            # tail is emitted one block late, split so the UT PSUM->SBUF copy
            # (ACT) gets PE work between it and the Wv matmuls that read it.
            def emit_scores(j, ci, c0, w, sums):
                ps_s = pmm.tile([P, 512], FP32, tag="mm")
                for dt in range(DT):
                    nc.tensor.matmul(ps_s[:, :w], QT[:, dt, j * P:(j + 1) * P],
                                     XT[:, dt, c0:c0 + w],
                                     start=(dt == 0), stop=(dt == DT - 1))
                exps = work.tile([P, 512], BF16, tag="exps")
                nc.scalar.activation(exps[:, :w], ps_s[:, :w],
                                     mybir.ActivationFunctionType.Exp,
                                     scale=1.0 / 32.0)
                if c0 + w == (2 * j + 2) * P:  # final two kv blocks: mask
                    nc.vector.tensor_mul(exps[:, w - 256:w],
                                         exps[:, w - 256:w], mask_sb[:])
                csum = small.tile([P, 1], FP32, tag="csum")
                nc.vector.tensor_reduce(csum[:], exps[:, :w],
                                        axis=mybir.AxisListType.X,
                                        op=mybir.AluOpType.add)
                nc.vector.tensor_add(sums[:], sums[:], csum[:])
                return exps

            def emit_transpose(j, ci, exps, expsT, c0, w):
                ps_t = pt.tile([P, 512], BF16, tag="pt")
                nblk = w // P
                for i in range(nblk):
                    nc.tensor.transpose(ps_t[:, i * P:(i + 1) * P],
                                        exps[:, i * P:(i + 1) * P], ident[:])
                nc.scalar.copy(expsT[:, c0:c0 + w], ps_t[:, :w])

            def emit_ctx(j, UT, expsT):
                # One PSUM accumulation chain at a time per bank: iterate
                # d-blocks in the outer loop so each chain opens and closes
                # before the next begins.
                n_kb = 2 * j + 2
                for db in range(DT):
                    for kb in range(n_kb):
                        nc.tensor.matmul(UT[:, db, :],
                                         XN[:, kb, db * P:(db + 1) * P],
                                         expsT[:, kb * P:(kb + 1) * P],
                                         start=(kb == 0), stop=(kb == n_kb - 1))

            def emit_tail_pre(UT, expsT, sums, j):
                recip = small.tile([P, 1], FP32, tag="recip")
                nc.vector.reciprocal(recip[:], sums[:])
                ut_sb = work.tile([P, DT, P], BF16, tag="ut")
                nc.scalar.copy(ut_sb[:], UT[:])
                return ut_sb, recip, j

            def emit_tail_post(ut_sb, recip, j):
                out_sb = work.tile([P, E], FP32, tag="out")
                for eh in range(2):
                    ps_o = pmm.tile([P, 512], FP32, tag="mm")
                    for dt in range(DT):
                        nc.tensor.matmul(ps_o[:], ut_sb[:, dt, :],
                                         wv_sb[:, dt, eh * 512:(eh + 1) * 512],
                                         start=(dt == 0), stop=(dt == DT - 1))
                    nc.scalar.activation(out_sb[:, eh * 512:(eh + 1) * 512],
                                         ps_o[:],
                                         mybir.ActivationFunctionType.Copy,
                                         scale=recip[:])
                    nc.sync.dma_start(
                        out=out[j * P:(j + 1) * P, eh * 512:(eh + 1) * 512],
                        in_=out_sb[:, eh * 512:(eh + 1) * 512])

            def emit_ctx_chains(UT, expsT, j, dbs):
                # Sequential PSUM chains: each db's accumulation over all kv
                # blocks opens and closes before the next db starts (a PSUM
                # bank allows only one pending accumulation group).
                n_kb = 2 * j + 2
                for db in dbs:
                    for kb in range(n_kb):
                        nc.tensor.matmul(UT[:, db, :],
                                         XN[:, kb, db * P:(db + 1) * P],
                                         expsT[:, kb * P:(kb + 1) * P],
                                         start=(kb == 0), stop=(kb == n_kb - 1))

            prev = None     # (UT, expsT, sums, j): block awaiting ctx+tail_pre
            post = None     # (ut_sb, recip, j): block awaiting Wv + store
            for j in range(QB):
                widths = [512] * ((j + 1) // 2) + ([256] if j % 2 == 0 else [])
                n = len(widths)
                sums = small.tile([P, 1], FP32, tag="sums")
                nc.vector.memset(sums[:], 0.0)
                UT = pu.tile([P, DT, P], FP32, tag="pu")
                expsT = work.tile([P, T], BF16, tag="expsT")
                c0 = 0
                for ci, w in enumerate(widths):
                    exps = emit_scores(j, ci, c0, w, sums)
                    if ci == 0 and post is not None:
                        emit_tail_post(*post)
                        post = None
                    if prev is not None:
                        lo, hi = (DT * ci) // n, (DT * (ci + 1)) // n
                        emit_ctx_chains(prev[0], prev[1], prev[3], range(lo, hi))
                        if ci == n - 1:
                            post = emit_tail_pre(*prev)
                            prev = None
                    emit_transpose(j, ci, exps, expsT, c0, w)
                    c0 += w
                prev = (UT, expsT, sums, j)
            # flush: tail of block QB-2, then chains + tail of block QB-1
            emit_tail_post(*post)
            emit_ctx_chains(prev[0], prev[1], prev[3], range(DT))
            post = emit_tail_pre(*prev)
            emit_tail_post(*post)

    nc.compile()
    return nc


def _get_nc():
    global _CACHED_NC
    if _CACHED_NC is None:
        _CACHED_NC = _build()
    return _CACHED_NC


def _prep_inputs(x, Wq, Wk, Wv):
    bf = ml_dtypes.bfloat16
    tril = np.tril(np.ones((P, P), np.float32))
    ones = np.ones((P, P), np.float32)
    zeros = np.zeros((P, P), np.float32)
    # fold Wq/Wk into the query projection: scores = (x_q G) x_kv^T with
    # G = Wq @ Wk^T, so only this core's 1024 q tokens get projected and
    # raw x_kv columns serve as the K side of the score matmul.
    G = (np.asarray(Wq, np.float64) @ np.asarray(Wk, np.float64).T)
    G = G.astype(np.float32).astype(bf)
    # [p, b, (t, e')] tiling: gq[p, b, t*128+e'] = G[t*128+p, b*128+e']
    gq_b = np.ascontiguousarray(
        G.reshape(DT, P, DT, P).transpose(1, 2, 0, 3).reshape(P, DT, D))
    wv_b = np.asarray(Wv, np.float32).astype(bf)
    in_maps = []
    for core in range(NCORES):
        b, r = core // 2, core % 2
        xtc = np.ascontiguousarray(x[b].T.astype(np.float32)).astype(bf)
        xqc = np.ascontiguousarray(
            xtc.reshape(D, KVB, P)[:, r::2, :].reshape(D, QB * P))
        m = (np.concatenate([tril, zeros], axis=1) if r == 0
             else np.concatenate([ones, tril], axis=1)).astype(bf)
        in_maps.append({
            "gq": gq_b,
            "xq": xqc,
            "xt": xtc,
            "xn": np.ascontiguousarray(x[b].astype(np.float32)).astype(bf),
            "wv": wv_b,
            "mask": m,
        })
    return in_maps


def _assemble(results, x_shape):
    outp = np.empty(x_shape, np.float32)
    for core in range(NCORES):
        b, r = core // 2, core % 2
        co = results[core]["out"]
        for j in range(QB):
            g = 2 * j + r
            outp[b, g * P:(g + 1) * P, :] = co[j * P:(j + 1) * P, :]
    return outp


def kernel(x, Wq, Wk, Wv):
    assert x.shape == (B, T, D) and Wq.shape == (D, E)
    nc = _get_nc()
    in_maps = _prep_inputs(x, Wq, Wk, Wv)
    res = run_bass_kernel_spmd(nc, in_maps, core_ids=list(range(NCORES)))
    return _assemble(res.results, x.shape)


# revision 13
# speedup vs baseline: 1.1833x; 1.0444x over previous
"""Causal single-head attention on 8 Trainium2 NeuronCores.

Problem: x [4, 2048, 1024] fp32; Wq/Wk/Wv [1024, 1024] fp32.
  q/k/v = x @ W*; scores = q k^T / 32 (causal); out = softmax(scores) @ v.

Sharding: 8 cores = 4 batches x 2 roles. Within a batch, the 16
128-row q-blocks are split alternately: role r takes global blocks
g = 2j+r (j = 0..7) — this balances causal attention work between the
pair. Weight folding is done on the *query* side: with G = Wq Wk^T,
scores = (x_q G) x_kv^T, so each core projects Q~ = x_q @ G only for
its OWN 1024 q tokens (half the cost of projecting K for all 2048 kv
tokens) and contracts scores directly against raw x_kv^T columns.
The V projection is likewise folded past the attention contraction:
out = (attn @ x_kv) @ Wv, so attn@V runs against raw x rows and Wv is
applied to the [1024, 1024] normalized context afterwards. No
projection over the 2048 kv tokens happens anywhere — all per-core
matmul work is proportional to its 1024 q rows except the causal
score/context contractions themselves.

Input DMAs are chunked and ordered by first use (G block 0, then the
first half of x_q^T, ...) so the first Q~ matmul issues ~3.5 us in
instead of waiting ~35 us for the full input set (the DMA engines are
a serial resource at ~360 GB/s).

Each q-block's tail (normalize -> PE-transpose -> @Wv -> store) is
emitted one block late, software-pipelined under the next block's
score/context matmuls.

The program is SPMD-uniform: role differences live only in the
host-gathered inputs (xq = q-token columns of x^T in q-block order)
and in the [128, 256] mask applied to the last two kv blocks of each
padded row ([tril|zeros] for role 0, [ones|tril] for role 1).

Numerics: all matmuls in bf16 (inputs rounded on host) with fp32
PSUM accumulation; softmax in fp32 without max-subtraction (scores
are O(5), exp can't overflow), normalization deferred to after the
attn@V matmul. End-to-end max-abs error vs the fp32 reference is
~1e-2 of the output scale.
"""

import numpy as np
import ml_dtypes

import concourse.bass as bass
import concourse.bacc as bacc
import concourse.tile as tile
from concourse import bass_isa, mybir
from concourse.bass_utils import run_bass_kernel_spmd

P = 128
D = 1024          # d_in
E = 1024          # d_out
T = 2048          # seq len
B = 4             # batch
DT = D // P       # 8 d-tiles
QB = 8            # q blocks per core
KVB = T // P      # 16 kv blocks
NCORES = 8

FP32 = mybir.dt.float32
BF16 = mybir.dt.bfloat16

_CACHED_NC = None


def _build(pmm_bufs=3, pu_bufs=2, work_bufs=2):
    nc = bacc.Bacc(None, target_bir_lowering=False)
    # gq: G = Wq @ Wk^T pre-tiled on host as [p, out-dblock b, (d'tile t, e')]
    # so gq[:, b, t*128:(t+1)*128] is the lhsT for output block b, tile t.
    gq = nc.dram_tensor("gq", [P, DT, D], BF16, kind="ExternalInput")
    xq = nc.dram_tensor("xq", [D, QB * P], BF16, kind="ExternalInput")
    xt = nc.dram_tensor("xt", [D, T], BF16, kind="ExternalInput")
    xn = nc.dram_tensor("xn", [T, D], BF16, kind="ExternalInput")
    wv = nc.dram_tensor("wv", [D, E], BF16, kind="ExternalInput")
    mask = nc.dram_tensor("mask", [P, 2 * P], BF16, kind="ExternalInput")
    out = nc.dram_tensor("out", [QB * P, E], FP32, kind="ExternalOutput")

    xq_r = xq.rearrange("(dt p) t -> p dt t", p=P)
    xt_r = xt.rearrange("(dt p) t -> p dt t", p=P)
    xn_r = xn.rearrange("(tt p) d -> p tt d", p=P)
    wv_r = wv.rearrange("(dt p) e -> p dt e", p=P)

    with tile.TileContext(nc) as tc:
        with (
            tc.tile_pool(name="const", bufs=1) as const,
            tc.tile_pool(name="big", bufs=1) as big,
            tc.tile_pool(name="wpool", bufs=1) as wpool,
            tc.tile_pool(name="work", bufs=work_bufs) as work,
            tc.tile_pool(name="small", bufs=8) as small,
            tc.tile_pool(name="pmm", bufs=pmm_bufs, space="PSUM") as pmm,
            tc.tile_pool(name="pu", bufs=pu_bufs, space="PSUM") as pu,
        ):
            XT = big.tile([P, DT, T], BF16)       # raw x^T, d-tile major
            XN = big.tile([P, KVB, D], BF16)      # raw x rows, kv-tile major
            QT = big.tile([P, DT, QB * P], BF16)  # Q~^T (computed on-chip)
            gq_sb = wpool.tile([P, DT, D], BF16, tag="gq")
            xq_sb = wpool.tile([P, DT, QB * P], BF16, tag="xq")
            wv_sb = wpool.tile([P, DT, E], BF16, tag="wv")
            mask_sb = const.tile([P, 2 * P], BF16)

            # ---- input DMAs, ordered by first use (serial DMA resource)
            nc.sync.dma_start(out=gq_sb[:, 0, :], in_=gq[:, 0, :])
            nc.sync.dma_start(out=xq_sb[:, :, 0:256], in_=xq_r[:, :, 0:256])
            for b_ in range(1, DT):
                nc.sync.dma_start(out=gq_sb[:, b_, :], in_=gq[:, b_, :])
            nc.sync.dma_start(out=xq_sb[:, :, 256:512], in_=xq_r[:, :, 256:512])
            nc.sync.dma_start(out=xq_sb[:, :, 512:1024], in_=xq_r[:, :, 512:1024])
            nc.sync.dma_start(out=mask_sb[:], in_=mask[:, :])
            nc.sync.dma_start(out=XT[:, :, 0:1024], in_=xt_r[:, :, 0:1024])
            nc.sync.dma_start(out=XN[:, 0:KVB // 2, :], in_=xn_r[:, 0:KVB // 2, :])
            nc.sync.dma_start(out=wv_sb[:], in_=wv_r[:, :, :])
            nc.sync.dma_start(out=XT[:, :, 1024:2048], in_=xt_r[:, :, 1024:2048])
            nc.sync.dma_start(out=XN[:, KVB // 2:KVB, :], in_=xn_r[:, KVB // 2:KVB, :])

            # ---- Phase Q: Q~^T = G^T x_q^T over this core's 1024 q tokens
            for c, wq in [(0, 256), (256, 256), (512, 512)]:
                for b_ in range(DT):
                    ps = pmm.tile([P, 512], FP32, tag="mm")
                    for t in range(DT):
                        nc.tensor.matmul(ps[:, :wq], gq_sb[:, b_, t * P:(t + 1) * P],
                                         xq_sb[:, t, c:c + wq],
                                         start=(t == 0), stop=(t == DT - 1))
                    nc.scalar.copy(QT[:, b_, c:c + wq], ps[:, :wq])

            # ---- Phase C: attention per q block.
            #
            # Scores are computed TRANSPOSED (scoresT[kv, q], one sequential
            # PSUM chain per kv block), so the exp activation writes the
            # transposed softmax weights straight to SBUF -- no PE transposes
            # and no extra ACT copies. Softmax denominators become
            # partition-dim sums, computed on the otherwise-idle GPSIMD
            # engine (partition_all_reduce) plus small DVE row adds; the
            # reciprocal row is flipped to a per-partition column by a tiny
            # SBUF->SBUF DMA.
            #
            # Context accumulation is "UT-direct": UT[d, q] =
            # sum_kv XN[kv, d]^T expsT[kv, q], so the Wv matmul consumes it
            # as lhsT with no transpose. Normalization commutes with the Wv
            # matmul (per-q scalar) and is fused into the final PSUM->SBUF
            # copy as an ACT per-partition scale.
            #
            # Pipelining: block j's context chains are spread across block
            # j+1's chunks (each d-block chain opens and closes before the
            # next starts: a PSUM bank allows only one pending accumulation
            # group), and its Wv matmuls run early in block j+2.
            def emit_scores(j, c0, w, sums_row, expsT):
                ps_s = pmm.tile([P, 512], FP32, tag="mm")
                for i in range(w // P):
                    kb = c0 // P + i
                    for dt in range(DT):
                        nc.tensor.matmul(ps_s[:, i * P:(i + 1) * P],
                                         XT[:, dt, kb * P:(kb + 1) * P],
                                         QT[:, dt, j * P:(j + 1) * P],
                                         start=(dt == 0), stop=(dt == DT - 1))
                nc.scalar.activation(expsT[:, c0:c0 + w], ps_s[:, :w],
                                     mybir.ActivationFunctionType.Exp,
                                     scale=1.0 / 32.0)
                if c0 + w == (2 * j + 2) * P:  # final two kv blocks: mask
                    nc.vector.tensor_mul(expsT[:, c0 + w - 256:c0 + w],
                                         expsT[:, c0 + w - 256:c0 + w],
                                         mask_sb[:])
                ar = work.tile([P, 512], FP32, tag="allred")
                nc.gpsimd.partition_all_reduce(ar[:, :w], expsT[:, c0:c0 + w],
                                               channels=P,
                                               reduce_op=bass_isa.ReduceOp.add)
                for i in range(w // P):
                    nc.vector.tensor_add(sums_row[0:1, :], sums_row[0:1, :],
                                         ar[0:1, i * P:(i + 1) * P])

            def emit_ctx_chains(UT, expsT, j, dbs):
                n_kb = 2 * j + 2
                for db in dbs:
                    for kb in range(n_kb):
                        nc.tensor.matmul(UT[:, db, :],
                                         XN[:, kb, db * P:(db + 1) * P],
                                         expsT[:, kb * P:(kb + 1) * P],
                                         start=(kb == 0), stop=(kb == n_kb - 1))

            def emit_ut_copy(UT, ut_sb, half):
                nc.scalar.copy(ut_sb[:, half * 4:(half + 1) * 4, :],
                               UT[:, half * 4:(half + 1) * 4, :])

            def emit_wv(ut_sb, ps_os, dts):
                for eh in range(2):
                    for dt in dts:
                        nc.tensor.matmul(ps_os[eh][:], ut_sb[:, dt, :],
                                         wv_sb[:, dt, eh * 512:(eh + 1) * 512],
                                         start=(dt == 0), stop=(dt == DT - 1))

            def emit_store(ps_os, recip, j):
                out_sb = work.tile([P, E], FP32, tag="out")
                for eh in range(2):
                    nc.scalar.activation(out_sb[:, eh * 512:(eh + 1) * 512],
                                         ps_os[eh][:],
                                         mybir.ActivationFunctionType.Copy,
                                         scale=recip[:])
                    nc.sync.dma_start(
                        out=out[j * P:(j + 1) * P, eh * 512:(eh + 1) * 512],
                        in_=out_sb[:, eh * 512:(eh + 1) * 512])

            def emit_tail_post(ut_sb, recip, j):
                ps_o0 = pmm.tile([P, 512], FP32, tag="mm")
                ps_o1 = pmm.tile([P, 512], FP32, tag="mm")
                emit_wv(ut_sb, [ps_o0, ps_o1], range(DT))
                emit_store([ps_o0, ps_o1], recip, j)

            prev = None     # (UT, expsT, recip, j): block awaiting ctx chains
            post = None     # (ut_sb, recip, j): block awaiting Wv + store
            for j in range(QB):
                widths = [512] * ((j + 1) // 2) + ([256] if j % 2 == 0 else [])
                n = len(widths)
                sums_row = small.tile([1, P], FP32, tag="sumsrow")
                nc.vector.memset(sums_row[0:1, :], 0.0)
                UT = pu.tile([P, DT, P], FP32, tag="pu")
                expsT = work.tile([P, T], BF16, tag="expsT")
                c0 = 0
                for ci, w in enumerate(widths):
                    emit_scores(j, c0, w, sums_row, expsT)
                    if ci == 0 and post is not None:
                        emit_tail_post(*post)
                        post = None
                    if prev is not None:
                        lo, hi = (DT * ci) // n, (DT * (ci + 1)) // n
                        emit_ctx_chains(prev[0], prev[1], prev[3], range(lo, hi))
                        if ci == n - 1:
                            ut_sb = work.tile([P, DT, P], BF16, tag="ut")
                            for half in range(2):
                                emit_ut_copy(prev[0], ut_sb, half)
                            post = (ut_sb, prev[2], prev[3])
                            prev = None
                    c0 += w
                recip_row = small.tile([1, P], FP32, tag="reciprow")
                nc.vector.reciprocal(recip_row[0:1, :], sums_row[0:1, :])
                recip = small.tile([P, 1], FP32, tag="recip")
                nc.sync.dma_start(out=recip[:, 0:1], in_=recip_row[0:1, :])
                prev = (UT, expsT, recip, j)
            # ---- flush: tail of block QB-2, then chains + tail of QB-1,
            # with the UT copy halves pipelined under chains/Wv matmuls.
            emit_tail_post(*post)
            UT, expsT, recip, j = prev
            ut_sb = work.tile([P, DT, P], BF16, tag="ut")
            emit_ctx_chains(UT, expsT, j, range(4))
            emit_ut_copy(UT, ut_sb, 0)
            emit_ctx_chains(UT, expsT, j, range(4, DT))
            emit_ut_copy(UT, ut_sb, 1)
            ps_o0 = pmm.tile([P, 512], FP32, tag="mm")
            ps_o1 = pmm.tile([P, 512], FP32, tag="mm")
            emit_wv(ut_sb, [ps_o0, ps_o1], range(4))
            emit_wv(ut_sb, [ps_o0, ps_o1], range(4, DT))
            emit_store([ps_o0, ps_o1], recip, j)

    nc.compile()
    return nc


def _get_nc():
    global _CACHED_NC
    if _CACHED_NC is None:
        _CACHED_NC = _build()
    return _CACHED_NC


def _prep_inputs(x, Wq, Wk, Wv):
    bf = ml_dtypes.bfloat16
    tril = np.tril(np.ones((P, P), np.float32))
    ones = np.ones((P, P), np.float32)
    zeros = np.zeros((P, P), np.float32)
    # fold Wq/Wk into the query projection: scores = (x_q G) x_kv^T with
    # G = Wq @ Wk^T, so only this core's 1024 q tokens get projected and
    # raw x_kv columns serve as the K side of the score matmul.
    G = (np.asarray(Wq, np.float64) @ np.asarray(Wk, np.float64).T)
    G = G.astype(np.float32).astype(bf)
    # [p, b, (t, e')] tiling: gq[p, b, t*128+e'] = G[t*128+p, b*128+e']
    gq_b = np.ascontiguousarray(
        G.reshape(DT, P, DT, P).transpose(1, 2, 0, 3).reshape(P, DT, D))
    wv_b = np.asarray(Wv, np.float32).astype(bf)
    in_maps = []
    for core in range(NCORES):
        b, r = core // 2, core % 2
        xtc = np.ascontiguousarray(x[b].T.astype(np.float32)).astype(bf)
        xqc = np.ascontiguousarray(
            xtc.reshape(D, KVB, P)[:, r::2, :].reshape(D, QB * P))
        m_old = (np.concatenate([tril, zeros], axis=1) if r == 0
                 else np.concatenate([ones, tril], axis=1))
        # transposed per kv-block: mask[kv, block*128 + q] layout
        m = np.concatenate([m_old[:, 0:P].T, m_old[:, P:2 * P].T],
                           axis=1).astype(bf)
        in_maps.append({
            "gq": gq_b,
            "xq": xqc,
            "xt": xtc,
            "xn": np.ascontiguousarray(x[b].astype(np.float32)).astype(bf),
            "wv": wv_b,
            "mask": m,
        })
    return in_maps


def _assemble(results, x_shape):
    outp = np.empty(x_shape, np.float32)
    for core in range(NCORES):
        b, r = core // 2, core % 2
        co = results[core]["out"]
        for j in range(QB):
            g = 2 * j + r
            outp[b, g * P:(g + 1) * P, :] = co[j * P:(j + 1) * P, :]
    return outp


def kernel(x, Wq, Wk, Wv):
    assert x.shape == (B, T, D) and Wq.shape == (D, E)
    nc = _get_nc()
    in_maps = _prep_inputs(x, Wq, Wk, Wv)
    res = run_bass_kernel_spmd(nc, in_maps, core_ids=list(range(NCORES)))
    return _assemble(res.results, x.shape)
